# revision 47
# baseline (speedup 1.0000x reference)
"""Longformer-style BERT (banded + global attention), 2 layers, on 8 TRN2
NeuronCores via Bass/Tile. Sequence-parallel: each core owns 512 tokens.

Per-core scheme (T=512 local tokens, E=1024 extended key window):
  - embeddings + embedding-LN computed on HOST (the gather already is);
    device receives h0 bf16 per core.
  - residual h kept in SBUF bf16 (h_bf tiles); LN math fp32.
  - matmul activations: feature-major bf16 (hT, qT, kT, kgfT, qgT, kgT, oT).
  - v / vgf / vg: token-major bf16 with a ones-column per head so PV
    matmuls produce softmax denominators in psum column 64.
  - band attention: block-banded over 128-token tiles; static triangle
    masks on the edge tiles; invalid keys (out of range / global / pad)
    folded into v (zeroed rows) via kval01.
  - cross-core per layer (all collectives sized by OUTPUT in the fabric):
      AGG: AllGather of the <=GM own global-token h rows -> [NC_*GM, D]
      RS:  ReduceScatter(add) halo exchange: each core scatters its edge
           tiles into neighbor slots of a zero-initialized [S, D] buffer;
           the output chunk is exactly this core's 512 halo rows.
      RS2: ReduceScatter(add) of slot-ordered global-query partial sums
           [NC_*GM, H*65] -> each core gets its own globals' sums.
"""
import os
import sys

sys.path.insert(0, '/opt/trn_rl_repo')
sys.path.insert(0, os.path.dirname(os.path.abspath(__file__)))

import numpy as np
import ml_dtypes

import concourse.bass as bass
import concourse.tile as tile
from concourse import mybir
from concourse.bass_utils import run_bass_kernel_spmd

# ---- walrus sync-wait-limit workaround (inlined) ----
"""Workarounds for the pinned walrus build's per-instruction sync-wait limit.

This walrus errors with 'Too many sync wait commands' when an instruction
carries more than one sem wait. Two patches:

1. TileContext._lower_ordered_insts — before lowering, split any instruction
   with >MAXW on_wait entries: excess waits move to InstNoOp instructions
   inserted just before it on the same engine (engines are in-order, so
   waiting earlier on the same engine is always sound).

2. TileContext._drain_and_barrier — the end-of-kernel drain gets its waits
   spread over SP nops the same way.
"""
import concourse.tile as _tile
from concourse import mybir as _mybir
from concourse.vector_clock import ScopedClock as _ScopedClock

_MAXW = 1


def _split_waits_in_ordered(tc, ordered):
    nc = tc.nc
    for bb_name, insts in ordered.items():
        new_list = []
        for inst in insts:
            si = inst.sync_info
            waits = list(si.on_wait) if si is not None and si.on_wait else []
            if len(waits) > _MAXW and inst.engine != _mybir.EngineType.Unassigned:
                keep = waits[:_MAXW]
                extra = waits[_MAXW:]
                for j in range(0, len(extra), _MAXW):
                    nop = _mybir.InstNoOp(
                        name=nc.get_next_instruction_name(),
                        engine=inst.engine,
                        ins=[],
                        outs=[],
                        sync_info=_mybir.SyncInfo(
                            on_wait=extra[j:j + _MAXW], on_update=[]
                        ),
                        bass_nofuse=True,
                    )
                    nc.register_instruction(nop, overwrite=True)
                    new_list.append(nop)
                inst.sync_info = _mybir.SyncInfo(
                    on_wait=keep,
                    on_update=list(si.on_update) if si.on_update else [],
                )
            new_list.append(inst)
        ordered[bb_name] = new_list


_orig_lower = _tile.TileContext._lower_ordered_insts


def _patched_lower(self, ordered):
    _split_waits_in_ordered(self, ordered)
    return _orig_lower(self, ordered)


_tile.TileContext._lower_ordered_insts = _patched_lower


def _patched_drain_and_barrier(self, tick_clock, wait_clock):
    nc = self.nc
    drain_inst = nc.sync.drain()
    wait_clock.add_sem_waits(
        drain_inst.ins, _ScopedClock({None: tick_clock.global_clock})
    )
    si = drain_inst.ins.sync_info
    waits = list(si.on_wait) if si is not None and si.on_wait else []
    if len(waits) > _MAXW:
        drain_inst.ins.sync_info = _mybir.SyncInfo(
            on_wait=waits[:_MAXW],
            on_update=list(si.on_update) if si.on_update else [],
        )
        for i in range(_MAXW, len(waits), _MAXW):
            nop = nc.sync.nop(nofuse=True)
            nsi = nop.ins.sync_info
            nop.ins.sync_info = _mybir.SyncInfo(
                on_wait=waits[i:i + _MAXW],
                on_update=(list(nsi.on_update)
                           if (nsi is not None and nsi.on_update) else []),
            )
    nc.all_engine_barrier()
    assert self.sems is not None
    popped = nc._tile_sem_poison_stack.pop()
    assert popped is self._sem_poison
    nc.clear_and_free_semaphores(list(self.sems.allocated().values()))
    nc.all_engine_barrier()


_tile.TileContext._drain_and_barrier = _patched_drain_and_barrier


F32 = mybir.dt.float32
BF16 = mybir.dt.bfloat16
I32 = mybir.dt.int32
AF = mybir.ActivationFunctionType
AX = mybir.AxisListType
OP = mybir.AluOpType

NC_ = 8           # cores
S = 4096
D = 768
H = 12
FF = 3072
L = 2
T = S // NC_      # 512 tokens per core
QT = T // 128     # 4 query tiles per core
DT = D // 128     # 6 feature tiles
FT = FF // 128    # 24 ff tiles
ET = QT + 4       # 8 extended key tiles (halo 2 each side)
E = ET * 128      # 1024
GM = 16           # padded per-core global-token slots
NS = NC_ * GM     # 128 global slots total
SCALE = 1.0 / 8.0
NEG = -30.0
EPS = 1e-5

bfd = ml_dtypes.bfloat16


# ----------------------------------------------------------------------------
# device program
# ----------------------------------------------------------------------------

def build_program():
    nc = bass.Bass()

    def inp(name, shape, dtype=F32):
        return nc.declare_dram_parameter(name, list(shape), dtype,
                                         isOutput=False)

    t = {}
    t["h0"] = inp("h0", [T, D], BF16)
    for w in ("Wq", "Wk", "Wv", "Wqg", "Wkg", "Wvg", "Wo"):
        t[w] = inp(w, [L, D, D], BF16)
    t["Wf1"] = inp("Wf1", [L, D, FF], BF16)
    t["Wf2"] = inp("Wf2", [L, FF, D], BF16)
    for b in ("bq_p", "bk_p", "bkg_p", "bqg_p"):
        t[b] = inp(b, [L, 128, DT])
    t["bf1_p"] = inp("bf1_p", [L, 128, FT])
    for b in ("bv_b", "bvg_b", "bo_b", "bf2_b"):
        t[b] = inp(b, [L, 128, D], BF16)
    for b in ("ln1s_b", "ln1b_b", "ln2s_b", "ln2b_b"):
        t[b] = inp(b, [L, 128, D], BF16)
    t["rs_idx"] = inp("rs_idx", [128, 4], I32)
    t["agg_idx"] = inp("agg_idx", [128, 4], I32)
    t["kval_bias"] = inp("kval_bias", [128, ET])
    t["kval01"] = inp("kval01", [128, ET])
    t["gkey_bias"] = inp("gkey_bias", [NS, 1])
    t["fkey_bias"] = inp("fkey_bias", [128, QT])
    t["glb1m"] = inp("glb1m", [128, QT])
    t["sel"] = inp("sel", [QT, GM, 128], BF16)
    t["tri_lo"] = inp("tri_lo", [128, 128], BF16)
    t["tri_hi"] = inp("tri_hi", [128, 128], BF16)
    t["ident"] = inp("ident", [128, 128], BF16)
    t["out"] = nc.declare_dram_parameter("out", [T, D], F32, isOutput=True)

    with tile.TileContext(nc) as tc:
        with (
            tc.tile_pool(name="cn", bufs=1) as cn,
            tc.tile_pool(name="wp", bufs=1) as wp,
            tc.tile_pool(name="act", bufs=1) as act,
            tc.tile_pool(name="scr", bufs=1) as scr,
            tc.tile_pool(name="pTp", bufs=1) as pTp,
            tc.tile_pool(name="psp", bufs=1, space="PSUM") as psp,
            tc.tile_pool(name="dram", bufs=1, space="DRAM") as dram,
        ):
            _body(nc, t, cn, wp, act, scr, pTp, psp, dram)
    return nc


def _body(nc, t, cn, wp, act, scr, pTp, psp, dram):
    def load_const(name, shape, dtype=F32):
        tl = cn.tile(list(shape), dtype, tag=name, name=name + "_sb")
        nc.sync.dma_start(tl[:], t[name][:])
        return tl

    tri_lo = load_const("tri_lo", [128, 128], BF16)
    tri_hi = load_const("tri_hi", [128, 128], BF16)
    ident = load_const("ident", [128, 128], BF16)
    rs_idx = load_const("rs_idx", [128, 4], I32)
    agg_idx = load_const("agg_idx", [128, 4], I32)
    kval_bias = load_const("kval_bias", [128, ET])
    kval01 = load_const("kval01", [128, ET])
    gkey_bias = load_const("gkey_bias", [NS, 1])
    fkey_bias = load_const("fkey_bias", [128, QT])
    glb1m = load_const("glb1m", [128, QT])
    sel_sb = cn.tile([GM, QT, 128], BF16, tag="sel", name="sel_sb")
    nc.sync.dma_start(sel_sb[:], t["sel"].rearrange("q g t -> g q t")[:])
    eps_c = cn.tile([128, 1], F32, tag="eps_c", name="eps_c")
    nc.vector.memset(eps_c[:], EPS)

    def big32(name="b32"):
        return scr.tile([128, D], F32, tag="sD32", bufs=3, name=name)

    def small32(name="s32"):
        return scr.tile([128, 1], F32, tag="s1", bufs=6, name=name)

    def bigbf(name="bbf"):
        return scr.tile([128, D], BF16, tag="sDbf", bufs=2, name=name)

    # ---- layernorm: x fp32 [128, D] -> out_ap; var = E[x^2] - mean^2
    def layer_norm(x_tile, s_b, b_b, out_ap):
        red = small32("ln_red")
        nc.vector.tensor_reduce(red[:], x_tile[:], axis=AX.X, op=OP.add)
        mean = small32("ln_mean")
        nc.scalar.mul(mean[:], red[:], 1.0 / D)
        sq = scr.tile([128, D], F32, tag="ln_sq", bufs=1, name="ln_sq")
        ssq = small32("ln_ssq")
        nc.scalar.activation(sq[:], x_tile[:], AF.Square,
                             accum_out=ssq[:, 0:1])
        # bias = eps - mean^2
        vb = small32("ln_vb")
        nc.vector.tensor_scalar(vb[:], mean[:], mean[:, 0:1], -1.0,
                                op0=OP.mult, op1=OP.mult)
        nc.vector.tensor_scalar_add(vb[:], vb[:], eps_c[:, 0:1])
        std = small32("ln_std")
        nc.scalar.activation(std[:], ssq[:], AF.Sqrt, bias=vb[:, 0:1],
                             scale=1.0 / D)
        rstd = small32("ln_rstd")
        nc.vector.reciprocal(rstd[:], std[:])
        zn = big32("ln_zn")
        nc.vector.tensor_scalar(zn[:], x_tile[:], mean[:, 0:1],
                                rstd[:, 0:1], op0=OP.subtract, op1=OP.mult)
        tmp = big32("ln_tmp")
        nc.vector.tensor_mul(tmp[:], zn[:], s_b[:])
        nc.vector.tensor_add(out_ap, tmp[:], b_b[:])

    h1 = [act.tile([128, D], F32, tag=f"h1_{i}", name=f"h1_{i}")
          for i in range(QT)]
    h_bf = [act.tile([128, D], BF16, tag=f"hbf_{i}", name=f"hbf_{i}")
            for i in range(QT)]

    # ---- one-time zero init of collective input buffers; h0 load
    zbf = cn.tile([128, D], BF16, tag="zbf", name="zbf")
    nc.vector.memset(zbf[:], 0.0)
    rs_in = dram.tile([S, D], BF16, tag="rs_in", name="rs_in")
    engs = [nc.sync, nc.scalar, nc.gpsimd]
    for b in range(S // 128):
        engs[b % 3].dma_start(rs_in[b * 128:(b + 1) * 128, :], zbf[:])
    agg_in = dram.tile([GM, D], BF16, tag="agg_in", name="agg_in")
    nc.scalar.dma_start(agg_in[:], zbf[0:GM, :])
    for i in range(QT):
        nc.sync.dma_start(h_bf[i][:], t["h0"][i * 128:(i + 1) * 128, :])

    # ---------------- layers ----------------
    for l in range(L):
        # ---- C1: globals AllGather first (gates the global-query path)
        agg_out = dram.tile([NS, D], BF16, tag="agg_out",
                            name=f"agg_out{l}", addr_space="Shared")
        for i in range(QT):
            nc.gpsimd.indirect_dma_start(
                out=agg_in[:], out_offset=bass.IndirectOffsetOnAxis(
                    ap=agg_idx[:, i:i + 1], axis=0),
                in_=h_bf[i][:], in_offset=None,
                bounds_check=GM - 1, oob_is_err=False,
            )
        nc.gpsimd.collective_compute(
            "AllGather", OP.bypass,
            ins=[agg_in[:]], outs=[agg_out[:]],
            replica_groups=[list(range(NC_))],
        )
        # ---- C2: halo ReduceScatter
        rs_out = dram.tile([2 * 256, D], BF16, tag="rs_out",
                           name=f"rs_out{l}")
        for i in range(QT):
            nc.gpsimd.indirect_dma_start(
                out=rs_in[:], out_offset=bass.IndirectOffsetOnAxis(
                    ap=rs_idx[:, i:i + 1], axis=0),
                in_=h_bf[i][:], in_offset=None,
                bounds_check=S - 1, oob_is_err=False,
            )
        nc.gpsimd.collective_compute(
            "ReduceScatter", OP.add,
            ins=[rs_in[:]], outs=[rs_out[:]],
            replica_groups=[list(range(NC_))],
        )

        # ---- hT_own via PE transposes
        hT_own = act.tile([128, DT, T], BF16, tag="hT_own", name=f"hTo{l}")
        for i in range(QT):
            for d in range(DT):
                tps = psp.tile([128, 128], BF16, tag="p", bufs=8, name="trh")
                nc.tensor.transpose(tps[:], h_bf[i][:, d * 128:(d + 1) * 128],
                                    ident[:])
                nc.vector.tensor_copy(hT_own[:, d, i * 128:(i + 1) * 128],
                                      tps[:])

        def wslab(src_ap, name, eng=None):
            tl = wp.tile([128, DT, D], BF16, tag="w", bufs=7, name=name)
            (eng or nc.sync).dma_start(
                tl[:], src_ap.rearrange("(k p) o -> p k o", p=128)[:])
            return tl

        w_kg = wslab(t["Wkg"][l], f"wkg{l}")
        w_vg = wslab(t["Wvg"][l], f"wvg{l}", eng=nc.scalar)
        w_q = wslab(t["Wq"][l], f"wq{l}")
        w_qg = wslab(t["Wqg"][l], f"wqg{l}", eng=nc.scalar)
        w_k = wslab(t["Wk"][l], f"wk{l}")
        w_v = wslab(t["Wv"][l], f"wv{l}", eng=nc.scalar)
        w_o = wslab(t["Wo"][l], f"wo{l}")

        def bload(name, n=DT, dtype=F32):
            tl = wp.tile([128, n], dtype, tag=f"b_{name}", name=f"{name}{l}")
            nc.scalar.dma_start(tl[:], t[name][l][:])
            return tl

        b_q = bload("bq_p")
        b_k = bload("bk_p")
        b_kg = bload("bkg_p")
        b_qg = bload("bqg_p")
        b_f1 = bload("bf1_p", FT)
        b_v = bload("bv_b", D, BF16)
        b_vg = bload("bvg_b", D, BF16)
        b_o = bload("bo_b", D, BF16)
        b_f2 = bload("bf2_b", D, BF16)
        ln1s = bload("ln1s_b", D, BF16)
        ln1b = bload("ln1b_b", D, BF16)
        ln2s = bload("ln2s_b", D, BF16)
        ln2b = bload("ln2b_b", D, BF16)

        # ---- projections; rhs given as chunks (tile, out_col0, width)
        def proj_chunk(w_sb, b_sb, o, rhs, col0, w):
            for cc in range(0, w, 512):
                cw = min(cc + 512, w) - cc
                for ot in range(DT):
                    ps = psp.tile([128, 512], F32, tag="p", bufs=8,
                                  name="pw")
                    for k in range(DT):
                        nc.tensor.matmul(
                            ps[:, 0:cw],
                            w_sb[:, k, ot * 128:(ot + 1) * 128],
                            rhs[:, k, cc:cc + cw],
                            start=(k == 0), stop=(k == DT - 1))
                    nc.vector.tensor_scalar_add(
                        o[:, ot, col0 + cc:col0 + cc + cw], ps[:, 0:cw],
                        b_sb[:, ot:ot + 1])

        def proj_tm_tiles(w_sb, b_bc, o, tts, rhs_of):
            for tt in tts:
                rhs, tcol = rhs_of(tt)
                for c0 in (0, 512):
                    c1 = min(c0 + 512, D)
                    ps = psp.tile([128, 512], F32, tag="p", bufs=8, name="pt")
                    for k in range(DT):
                        nc.tensor.matmul(
                            ps[:, 0:c1 - c0],
                            rhs[:, k, tcol:tcol + 128],
                            w_sb[:, k, c0:c1],
                            start=(k == 0), stop=(k == DT - 1))
                    biased = big32("ptb")
                    nc.vector.tensor_add(biased[:, 0:c1 - c0],
                                         ps[:, 0:c1 - c0], b_bc[:, c0:c1])
                    nh = (c1 - c0) // 64
                    h0_ = c0 // 64
                    dst = o[:, tt].rearrange("p (hh c) -> p hh c", c=65)
                    nc.vector.tensor_copy(
                        dst[:, h0_:h0_ + nh, 0:64],
                        biased[:, 0:c1 - c0]
                        .rearrange("p (hh c) -> p hh c", c=64)[:])
                nc.vector.memset(
                    o[:, tt].rearrange("p (hh c) -> p hh c", c=65)
                    [:, :, 64:65], 1.0)

        # ---- AR-critical local projections first
        kgfT = act.tile([128, DT, T], BF16, tag="kgfT", name=f"kgfT{l}")
        proj_chunk(w_kg, b_kg, kgfT, hT_own, 0, T)
        vgf_sb = act.tile([128, QT, H * 65], BF16, tag="vgf_sb",
                          name=f"vgf{l}")
        proj_tm_tiles(w_vg, b_vg, vgf_sb, list(range(QT)),
                      lambda tt: (hT_own, tt * 128))
        qT = act.tile([128, DT, T], BF16, tag="qT", name=f"qT{l}")
        proj_chunk(w_q, b_q, qT, hT_own, 0, T)

        # ---- global-token path (after AGG): hgT, qg/kg/vg, partial
        # global-query attention, then RS2 of slot-ordered sums
        hg_tm = bigbf("hg_tm")
        nc.sync.dma_start(hg_tm[:], agg_out[:])
        hgT = act.tile([128, DT, NS], BF16, tag="hgT", name=f"hgT{l}")
        for d in range(DT):
            tps = psp.tile([128, 128], BF16, tag="p", bufs=8, name="trg")
            nc.tensor.transpose(tps[:], hg_tm[:, d * 128:(d + 1) * 128],
                                ident[:])
            nc.vector.tensor_copy(hgT[:, d, :], tps[:])

        def proj_fm_g(w_sb, b_sb, tag):
            o = act.tile([128, DT, NS], BF16, tag=tag, name=tag + str(l))
            for ot in range(DT):
                ps = psp.tile([128, 512], F32, tag="p", bufs=8, name="pg_")
                for k in range(DT):
                    nc.tensor.matmul(
                        ps[:, 0:NS], w_sb[:, k, ot * 128:(ot + 1) * 128],
                        hgT[:, k, :],
                        start=(k == 0), stop=(k == DT - 1))
                nc.vector.tensor_scalar_add(o[:, ot, :], ps[:, 0:NS],
                                            b_sb[:, ot:ot + 1])
            return o

        qgT = proj_fm_g(w_qg, b_qg, "qgT")
        kgT = proj_fm_g(w_k, b_k, "kgT")

        vg_sb = act.tile([NS, H * 65], BF16, tag="vg_sb", name=f"vg{l}")
        for c0 in (0, 512):
            c1 = min(c0 + 512, D)
            ps = psp.tile([128, 512], F32, tag="p", bufs=8, name="pvg")
            for k in range(DT):
                nc.tensor.matmul(ps[:, 0:c1 - c0], hgT[:, k, :],
                                 w_v[:, k, c0:c1],
                                 start=(k == 0), stop=(k == DT - 1))
            biased = big32("vgb")
            nc.vector.tensor_add(biased[:, 0:c1 - c0], ps[:, 0:c1 - c0],
                                 b_v[:, c0:c1])
            nh = (c1 - c0) // 64
            h0_ = c0 // 64
            dst = vg_sb.rearrange("p (hh c) -> p hh c", c=65)
            nc.vector.tensor_copy(
                dst[:, h0_:h0_ + nh, 0:64],
                biased[:, 0:c1 - c0]
                .rearrange("p (hh c) -> p hh c", c=64)[:])
        nc.vector.memset(
            vg_sb.rearrange("p (hh c) -> p hh c", c=65)[:, :, 64:65], 1.0)

        # ---- global-query attention partials (slot-ordered) + RS2
        stag = scr.tile([NS, H, 65], F32, tag="gq_stage", name=f"stag{l}")
        for hh in range(H):
            hp, hr = hh // 2, (hh % 2) * 64
            prow = slice(hr, hr + 64)
            pfs = []
            for kt in range(QT):
                sps = psp.tile([128, 128], F32, tag="p", bufs=8, name="sf")
                nc.tensor.matmul(
                    sps[:, 0:NS], kgfT[prow, hp, kt * 128:(kt + 1) * 128],
                    qgT[prow, hp, :], start=True, stop=True)
                pf = pTp.tile([128, NS], BF16, tag="pf", bufs=5, name="pf")
                nc.scalar.activation(pf[:], sps[:, 0:NS], AF.Exp,
                                     bias=fkey_bias[:, kt:kt + 1],
                                     scale=SCALE)
                pfs.append(pf)
            gps = psp.tile([128, 65], F32, tag="p", bufs=8, name="gps")
            for kt in range(QT):
                nc.tensor.matmul(gps[:, :], pfs[kt][:],
                                 vgf_sb[:, kt, hh * 65:(hh + 1) * 65],
                                 start=(kt == 0), stop=(kt == QT - 1))
            nc.vector.tensor_copy(stag[:, hh, :], gps[:, :])
        rs2_in = dram.tile([NS, H * 65], F32, tag="rs2_in", name=f"r2i{l}")
        rs2_out = dram.tile([GM, H * 65], F32, tag="rs2_out",
                            name=f"r2o{l}")
        nc.sync.dma_start(rs2_in[:], stag.rearrange("p a b -> p (a b)")[:])
        nc.gpsimd.collective_compute(
            "ReduceScatter", OP.add,
            ins=[rs2_in[:]], outs=[rs2_out[:]],
            replica_groups=[list(range(NC_))],
        )
        gsum = scr.tile([GM, H, 65], F32, tag="gq_sum", name=f"gsum{l}")
        nc.sync.dma_start(gsum.rearrange("p a b -> p (a b)")[:], rs2_out[:])

        # ---- band-side projections: k/v center, then halo after RS
        kT = act.tile([128, DT, E], BF16, tag="kT", name=f"kT{l}")
        proj_chunk(w_k, b_k, kT, hT_own, 256, T)
        v_sb = act.tile([128, ET, H * 65], BF16, tag="v_sb", name=f"v{l}")

        def v_rhs(tt):
            if tt < 2:
                return hT_hl, tt * 128
            if tt < 6:
                return hT_own, (tt - 2) * 128
            return hT_hr, (tt - 6) * 128

        hT_hl = act.tile([128, DT, 256], BF16, tag="hT_hl", name=f"hl{l}")
        hT_hr = act.tile([128, DT, 256], BF16, tag="hT_hr", name=f"hr{l}")
        proj_tm_tiles(w_v, b_v, v_sb, [2, 3, 4, 5], v_rhs)

        # ---- halo tiles from rs_out (plain DMA + PE transpose)
        for g in range(4):  # 0,1 left; 2,3 right
            htmp = bigbf(f"halo{g}")
            nc.sync.dma_start(htmp[:], rs_out[g * 128:(g + 1) * 128, :])
            dst, off = (hT_hl, g * 128) if g < 2 else (hT_hr, (g - 2) * 128)
            for d in range(DT):
                tps = psp.tile([128, 128], BF16, tag="p", bufs=8, name="trp")
                nc.tensor.transpose(tps[:], htmp[:, d * 128:(d + 1) * 128],
                                    ident[:])
                nc.vector.tensor_copy(dst[:, d, off:off + 128], tps[:])

        # ---- halo-dependent projection parts
        proj_chunk(w_k, b_k, kT, hT_hl, 0, 256)
        proj_chunk(w_k, b_k, kT, hT_hr, 768, 256)
        proj_tm_tiles(w_v, b_v, v_sb, [0, 1, 6, 7], v_rhs)
        # fold key-validity masking into v (zero rows + ones entries of
        # invalid keys) so band exps need no per-key bias
        for e in range(ET):
            nc.vector.tensor_scalar_mul(v_sb[:, e, :], v_sb[:, e, :],
                                        kval01[:, e:e + 1])

        # ---- band + global-key attention -> o_sb
        o_sb = act.tile([128, QT, D], BF16, tag="o_sb", name=f"osb{l}")
        for qt in range(QT):
            qsl = slice(qt * 128, (qt + 1) * 128)
            for hh in range(H):
                hp, hr = hh // 2, (hh % 2) * 64
                prow = slice(hr, hr + 64)
                sg = psp.tile([128, 128], F32, tag="p", bufs=8, name="sg")
                nc.tensor.matmul(sg[:, :], kgT[prow, hp, :],
                                 qT[prow, hp, qsl], start=True, stop=True)
                pg = pTp.tile([NS, 128], BF16, tag="pg", bufs=2, name="pg")
                nc.scalar.activation(pg[:], sg[:, :], AF.Exp,
                                     bias=gkey_bias[:, 0:1], scale=SCALE)
                sp4 = psp.tile([128, 512], F32, tag="p", bufs=8,
                               name="sp4")
                for a in range(4):
                    e = qt + a
                    nc.tensor.matmul(
                        sp4[:, a * 128:(a + 1) * 128],
                        kT[prow, hp, e * 128:(e + 1) * 128],
                        qT[prow, hp, qsl], start=True, stop=True)
                sp_hi = psp.tile([128, 128], F32, tag="p", bufs=8,
                                 name="sp_hi")
                nc.tensor.matmul(
                    sp_hi[:], kT[prow, hp, (qt + 4) * 128:(qt + 5) * 128],
                    qT[prow, hp, qsl], start=True, stop=True)
                pt4 = pTp.tile([128, 512], BF16, tag="pt4", bufs=8,
                               name="pt4")
                nc.scalar.activation(pt4[:], sp4[:], AF.Exp, scale=SCALE)
                pt_hi = pTp.tile([128, 128], BF16, tag="pth", bufs=4,
                                 name="pth")
                nc.scalar.activation(pt_hi[:], sp_hi[:], AF.Exp, scale=SCALE)
                nc.vector.tensor_mul(pt4[:, 0:128], pt4[:, 0:128], tri_lo[:])
                nc.vector.tensor_mul(pt_hi[:], pt_hi[:], tri_hi[:])
                ops = psp.tile([128, 65], F32, tag="p", bufs=8, name="ops")
                nc.tensor.matmul(ops[:], pg[:],
                                 vg_sb[:, hh * 65:(hh + 1) * 65],
                                 start=True, stop=False)
                for a in range(4):
                    e = qt + a
                    nc.tensor.matmul(
                        ops[:], pt4[:, a * 128:(a + 1) * 128],
                        v_sb[:, e, hh * 65:(hh + 1) * 65],
                        start=False, stop=False)
                nc.tensor.matmul(
                    ops[:], pt_hi[:],
                    v_sb[:, qt + 4, hh * 65:(hh + 1) * 65],
                    start=False, stop=True)
                rec = small32("rec")
                nc.vector.reciprocal(rec[:], ops[:, 64:65])
                fac = small32("fac")
                nc.vector.tensor_mul(fac[:], rec[:], glb1m[:, qt:qt + 1])
                nc.vector.tensor_scalar_mul(
                    o_sb[:, qt, hh * 64:(hh + 1) * 64], ops[:, 0:64],
                    fac[:, 0:1])

        # ---- og from the RS2 result (own slots); scatter into o_sb
        og = act.tile([GM, D], BF16, tag="og", name=f"og{l}")
        for hh in range(H):
            rec = small32("grec")
            nc.vector.reciprocal(rec[0:GM, :], gsum[:, hh, 64:65])
            nc.vector.tensor_scalar_mul(og[:, hh * 64:(hh + 1) * 64],
                                        gsum[:, hh, 0:64], rec[0:GM, 0:1])
        for qt in range(QT):
            for c0 in (0, 512):
                c1 = min(c0 + 512, D)
                sc = psp.tile([128, 512], F32, tag="p", bufs=8, name="sc")
                nc.tensor.matmul(sc[:, 0:c1 - c0], sel_sb[:, qt, :],
                                 og[:, c0:c1], start=True, stop=True)
                nc.vector.tensor_add(o_sb[:, qt, c0:c1], o_sb[:, qt, c0:c1],
                                     sc[:, 0:c1 - c0])

        # ---- oT via PE transposes (shares qT slot)
        oT = act.tile([128, DT, T], BF16, tag="qT", name=f"oT{l}")
        for qt in range(QT):
            for d in range(DT):
                tps = psp.tile([128, 128], BF16, tag="p", bufs=8, name="tro")
                nc.tensor.transpose(
                    tps[:], o_sb[:, qt, d * 128:(d + 1) * 128], ident[:])
                nc.vector.tensor_copy(oT[:, d, qt * 128:(qt + 1) * 128],
                                      tps[:])

        # ---- Wo + residual + LN1 -> h1 (f32)
        for qt in range(QT):
            x1 = big32("x1")
            nc.vector.tensor_add(x1[:], h_bf[qt][:], b_o[:])
            for c0 in (0, 512):
                c1 = min(c0 + 512, D)
                ps = psp.tile([128, 512], F32, tag="p", bufs=8, name="pwo")
                for k in range(DT):
                    nc.tensor.matmul(
                        ps[:, 0:c1 - c0], oT[:, k, qt * 128:(qt + 1) * 128],
                        w_o[:, k, c0:c1],
                        start=(k == 0), stop=(k == DT - 1))
                nc.vector.tensor_add(x1[:, c0:c1], x1[:, c0:c1],
                                     ps[:, 0:c1 - c0])
            layer_norm(x1, ln1s, ln1b, h1[qt][:])

        # ---- h1T via PE transposes (shares kgfT slot)
        h1T = act.tile([128, DT, T], BF16, tag="kgfT", name=f"h1T{l}")
        for qt in range(QT):
            h1b = bigbf(f"h1b{qt}")
            nc.vector.tensor_copy(h1b[:], h1[qt][:])
            for d in range(DT):
                tps = psp.tile([128, 128], BF16, tag="p", bufs=8, name="trh1")
                nc.tensor.transpose(tps[:], h1b[:, d * 128:(d + 1) * 128],
                                    ident[:])
                nc.vector.tensor_copy(h1T[:, d, qt * 128:(qt + 1) * 128],
                                      tps[:])

        # ---- FFN: x2 accumulates in-place on h1 (f32)
        for qt in range(QT):
            nc.vector.tensor_add(h1[qt][:], h1[qt][:], b_f2[:])
        for half in range(2):
            f1a = wslab(t["Wf1"][l][:, half * 1536:half * 1536 + 768],
                        f"f1a{l}{half}", eng=nc.scalar)
            f1b = wslab(t["Wf1"][l][:, half * 1536 + 768:(half + 1) * 1536],
                        f"f1b{l}{half}", eng=nc.scalar)
            f2a = wslab(t["Wf2"][l][half * 1536:half * 1536 + 768, :],
                        f"f2a{l}{half}", eng=nc.scalar)
            f2b = wslab(t["Wf2"][l][half * 1536 + 768:(half + 1) * 1536, :],
                        f"f2b{l}{half}", eng=nc.scalar)
            gT = act.tile([128, FT // 2, T], BF16, tag="v_sb", bufs=1,
                          name=f"gT{l}{half}")
            for ft in range(FT // 2):
                fabs = half * (FT // 2) + ft
                slab = f1a if ft < 6 else f1b
                ps = psp.tile([128, 512], F32, tag="p", bufs=8, name="pf1")
                for k in range(DT):
                    nc.tensor.matmul(
                        ps[:], slab[:, k, (ft % 6) * 128:(ft % 6 + 1) * 128],
                        h1T[:, k, :],
                        start=(k == 0), stop=(k == DT - 1))
                nc.scalar.activation(gT[:, ft, :], ps[:], AF.Gelu_apprx_tanh,
                                     bias=b_f1[:, fabs:fabs + 1])
            for qt in range(QT):
                for c0 in (0, 512):
                    c1 = min(c0 + 512, D)
                    ps = psp.tile([128, 512], F32, tag="p", bufs=8,
                                  name="pf2")
                    for k in range(FT // 2):
                        slab = f2a if k < 6 else f2b
                        nc.tensor.matmul(
                            ps[:, 0:c1 - c0],
                            gT[:, k, qt * 128:(qt + 1) * 128],
                            slab[:, k % 6, c0:c1],
                            start=(k == 0), stop=(k == FT // 2 - 1))
                    nc.vector.tensor_add(h1[qt][:, c0:c1], h1[qt][:, c0:c1],
                                         ps[:, 0:c1 - c0])
        for qt in range(QT):
            if l + 1 < L:
                layer_norm(h1[qt], ln2s, ln2b, h_bf[qt][:])
            else:
                hout = big32("hout")
                layer_norm(h1[qt], ln2s, ln2b, hout[:])
                nc.sync.dma_start(t["out"][qt * 128:(qt + 1) * 128, :],
                                  hout[:])


# ----------------------------------------------------------------------------
# host side
# ----------------------------------------------------------------------------

_prog_cache = {}


def _get_program():
    if "nc" not in _prog_cache:
        _prog_cache["nc"] = build_program()
    return _prog_cache["nc"]


def _prep_maps(inputs):
    gi = {k: np.asarray(v) for k, v in inputs.items()}
    x = gi["x"][0]
    segs = gi["segs"][0]
    mask = gi["mask_src"][0] > 0
    clss = gi["clss"][0]

    is_glb = np.zeros(S, bool)
    is_glb[clss] = True

    def bcast(v, dt=np.float32):
        v = np.asarray(v, np.float32)
        return np.broadcast_to(v[None, :], (128, v.shape[0])).astype(dt)

    def part(v):
        return np.asarray(v, np.float32).reshape(-1, 128).T.copy()

    # host-side embeddings + embedding layernorm (the gather is host-side
    # anyway); device receives h0 in bf16
    emb = (gi["word_emb"][x].astype(np.float32)
           + gi["pos_emb"].astype(np.float32)
           + gi["type_emb"][segs].astype(np.float32))
    mu = emb.mean(-1, keepdims=True)
    var = ((emb - mu) ** 2).mean(-1, keepdims=True)
    h0 = ((emb - mu) / np.sqrt(var + EPS) * gi["ln_e_s"][None, :]
          + gi["ln_e_b"][None, :]).astype(bfd)

    shared = {
        "Wq": gi["Wq"].astype(bfd), "Wk": gi["Wk"].astype(bfd),
        "Wv": gi["Wv"].astype(bfd), "Wqg": gi["Wqg"].astype(bfd),
        "Wkg": gi["Wkg"].astype(bfd), "Wvg": gi["Wvg"].astype(bfd),
        "Wo": gi["Wo"].astype(bfd),
        "Wf1": gi["Wf1"].astype(bfd), "Wf2": gi["Wf2"].astype(bfd),
        "bq_p": np.stack([part(gi["bq"][l]) for l in range(L)]),
        "bk_p": np.stack([part(gi["bk"][l]) for l in range(L)]),
        "bkg_p": np.stack([part(gi["bkg"][l]) for l in range(L)]),
        "bqg_p": np.stack([part(gi["bqg"][l]) for l in range(L)]),
        "bf1_p": np.stack([part(gi["bf1"][l]) for l in range(L)]),
        "bv_b": np.stack([bcast(gi["bv"][l], bfd) for l in range(L)]),
        "bvg_b": np.stack([bcast(gi["bvg"][l], bfd) for l in range(L)]),
        "bo_b": np.stack([bcast(gi["bo"][l], bfd) for l in range(L)]),
        "bf2_b": np.stack([bcast(gi["bf2"][l], bfd) for l in range(L)]),
        "ln1s_b": np.stack([bcast(gi["ln1_s"][l], bfd) for l in range(L)]),
        "ln1b_b": np.stack([bcast(gi["ln1_b"][l], bfd) for l in range(L)]),
        "ln2s_b": np.stack([bcast(gi["ln2_s"][l], bfd) for l in range(L)]),
        "ln2b_b": np.stack([bcast(gi["ln2_b"][l], bfd) for l in range(L)]),
        "tri_lo": (np.arange(128)[:, None] >= np.arange(128)[None, :])
                    .astype(bfd),
        "tri_hi": (np.arange(128)[:, None] <= np.arange(128)[None, :])
                    .astype(bfd),
        "ident": np.eye(128, dtype=bfd),
    }

    # per-core unique global positions -> slot assignment (slot-ordered
    # global queries/keys; NS = NC_*GM slots total)
    uniq = {c: sorted({int(p) for p in clss if c * T <= p < (c + 1) * T})
            for c in range(NC_)}
    assert all(len(u) <= GM for u in uniq.values()), \
        f"more than {GM} globals on one core: {[len(u) for u in uniq.values()]}"
    slot_of = {}
    for c, u in uniq.items():
        for s, p in enumerate(u):
            slot_of[p] = c * GM + s
    # per-slot key bias: 0 for occupied+unmasked slots, NEG otherwise
    gkey = np.full(NS, NEG, np.float32)
    for p, sl in slot_of.items():
        if mask[p]:
            gkey[sl] = 0.0
    shared["gkey_bias"] = gkey.reshape(NS, 1)

    OOB = np.int32(1 << 20)
    maps = []
    for c in range(NC_):
        s0, s1 = c * T, (c + 1) * T
        toks = np.arange(s0, s1)
        ext = np.arange(s0 - 256, s1 + 256)
        ext_ok = (ext >= 0) & (ext < S)
        extc = np.clip(ext, 0, S - 1)
        kval = np.where(ext_ok & mask[extc] & ~is_glb[extc], 0.0, NEG)
        # rs scatter: tiles 0,1 (first 256 rows) -> slot c-1 rows 256:512;
        # tiles 2,3 (last 256) -> slot c+1 rows 0:256
        rs_idx = np.full((QT, 128), OOB, np.int32)
        r = np.arange(128)
        for i in range(QT):
            if i < 2 and c > 0:
                rs_idx[i] = (c - 1) * 512 + 256 + i * 128 + r
            elif i >= 2 and c < NC_ - 1:
                rs_idx[i] = (c + 1) * 512 + (i - 2) * 128 + r
        # agg scatter: own unique global rows -> local slot in agg_in
        agg_idx = np.full((QT, 128), OOB, np.int32)
        for s, p in enumerate(uniq[c]):
            agg_idx[(p - s0) // 128, (p - s0) % 128] = s
        # og scatter selector: own slot s -> token position
        sel = np.zeros((QT, GM, 128), np.float32)
        for s, p in enumerate(uniq[c]):
            sel[(p - s0) // 128, s, (p - s0) % 128] = 1.0
        m = {
            "h0": h0[s0:s1],
            "rs_idx": rs_idx.T.copy(),
            "agg_idx": agg_idx.T.copy(),
            "kval_bias": kval.astype(np.float32).reshape(ET, 128).T.copy(),
            "kval01": (kval == 0.0).astype(np.float32)
                        .reshape(ET, 128).T.copy(),
            "fkey_bias": np.where(mask[toks], 0.0, NEG).astype(np.float32)
                           .reshape(QT, 128).T.copy(),
            "glb1m": (~is_glb[toks]).astype(np.float32)
                       .reshape(QT, 128).T.copy(),
            "sel": sel.astype(bfd),
        }
        m.update(shared)
        maps.append(m)
    return maps


def kernel(**inputs):
    nc = _get_program()
    maps = _prep_maps(inputs)
    res = run_bass_kernel_spmd(nc, maps, list(range(NC_)))
    out = np.concatenate([res.results[c]["out"] for c in range(NC_)], axis=0)
    return out[None].astype(np.float32)
